# revision 47
# baseline (speedup 1.0000x reference)
"""GCN layer (PyG GCNConv + ReLU + LN + residual + LN) on 8 Trainium2 cores.

Math: out = LN2(x + LN1(relu(A_hat @ x @ W.T + b)))  with
A_hat = D^-1/2 (A+I) D^-1/2.  The per-edge weight factorizes
(norm_e = dinv[src]*dinv[dst]) and aggregation commutes with the linear
layer, so each core:
  - gathers raw x rows (bf16) for the edges whose dst it owns (dma_gather)
  - scatter-adds them into per-dst-tile accumulators via one-hot matmuls
    on the PE: S[k, n] = (n == dstloc_k) * norm_k  built by one fused DVE
    tensor_scalar; psumT[feat, node] += g_chunk.T @ S
  - applies W via a second matmul (psum2[node, feat] = aggT.T @ W.T)
  - runs the bias/relu/LN1/residual/LN2 chain on 512-wide tiles.

Host-side numpy does graph preprocessing only: degrees, edge partitioning
by dst, bucketing by src>>15 (int16 gather-index windows), padding to
128-edge chunks, and a static chunk schedule shared by all 8 cores.
"""

import sys

import numpy as np

sys.path.insert(0, "/opt/trn_rl_repo")

EPS = 1e-5


def _cfg_full():
    return dict(
        N=100000,  # nodes
        C=128,  # features
        NCORES=8,
        SUB=32768,  # int16 gather window (rows per sub-table)
        GRP=8,  # dst tiles per psum group
        QUEUES=4,  # SWDGE queues (desc-gen runs on a Q7 cpu pair per queue)
        GBUFS=24,  # in-flight gather tiles
        SBUFS=4,  # in-flight scatter-matrix tiles
        NEGPAD=1,  # per-cell gather instructions + trailing -1 padding
        HOSTS=1,  # host-precomputed scatter one-hot matrices (DMA'd in)
        SIMSAFE=0,  # memset gather tiles every batch (CoreSim NaN poisoning)
        ABUFS=6,  # PSUM accumulator tiles (3 groups in flight)
    )


def _derived(cfg):
    N, NCORES = cfg["N"], cfg["NCORES"]
    npc = N // NCORES  # nodes per core
    assert npc * NCORES == N
    ntile = -(-npc // 128)  # dst tiles per core
    npad = ntile * 128
    nb = -(-N // cfg["SUB"])  # src buckets
    ngrp = -(-ntile // cfg["GRP"])
    return npc, ntile, npad, nb, ngrp


def _plan(cfg, src, dst, norm, dinv):
    """Build the shared static schedule + per-core host arrays.

    Returns (sched, cores) where sched has the chunk->tile mapping shared
    by all cores and cores[c] has idx/norm/dstloc arrays for core c.
    """
    N, C, NCORES, SUB, GRP = (
        cfg["N"], cfg["C"], cfg["NCORES"], cfg["SUB"], cfg["GRP"])
    npc, ntile, npad, nb, ngrp = _derived(cfg)
    ncell = ntile * nb

    per_core = []
    counts = np.zeros((NCORES, ncell), dtype=np.int64)
    for c in range(NCORES):
        base = c * npc
        m = (dst >= base) & (dst < base + npc)
        es, ed, en = src[m], dst[m], norm[m]
        # self loops (weight dinv[v]^2) are NOT routed through the gather:
        # they get a dedicated per-tile chunk loaded by plain contiguous DMA
        t = (ed - base) >> 7
        bkt = es // SUB
        cell = t * nb + bkt
        counts[c] = np.bincount(cell, minlength=ncell)
        per_core.append((es, ed - base, en, cell))

    cap = counts.max(axis=0)  # per (tile,bucket) max edges over cores
    chunks_per_cell = -(-cap // 128)  # 0 if cell empty on all cores
    # chunk schedule: group -> bucket -> tile in group -> chunks
    chunk_tile = []  # global chunk -> tile id
    cell_slot0 = np.zeros(ncell, dtype=np.int64)  # cell -> first slot
    batches = []  # (bucket, slot0, nslots) per gather instruction
    groups = []  # list of lists of tile ids
    bmax = cfg.get("BMAX", 896)
    negpad = cfg.get("NEGPAD", 1)
    slot = 0
    for g in range(ngrp):
        tiles = list(range(g * GRP, min((g + 1) * GRP, ntile)))
        groups.append(tiles)
        # dedicated self-loop chunk per tile: own rows arrive by plain
        # contiguous DMA (marker b=-1), scatter matrix is diag(dinv^2)
        for t in tiles:
            chunk_tile.append(t)
            batches.append((g, -1, slot, 128))
            slot += 128
        for b in range(nb):
            s0 = slot
            for t in tiles:
                cell = t * nb + b
                nch = int(chunks_per_cell[cell])
                if nch == 0:
                    continue
                cell_slot0[cell] = slot
                chunk_tile.extend([t] * nch)
                slot += nch * 128
                if negpad:
                    # per-cell instructions trimmed to the max-core count:
                    # the Q7 idx-copy loop costs ~10ns/idx, so the tail of
                    # 128-padding is dropped statically and per-core padding
                    # is dropped at runtime via trailing -1 idxs
                    p = slot - nch * 128
                    rem = int(cap[cell])
                    while rem > 0:
                        ns = min(bmax, rem)
                        batches.append((g, b, p, ns))
                        p += ns
                        rem -= ns
            if not negpad:
                # split into gather instructions of <= bmax indices (the
                # SWDGE descriptor carveout rejects much larger ones)
                p = s0
                while p < slot:
                    ns = min(bmax, slot - p)
                    batches.append((g, b, p, ns))
                    p += ns
    nslot = slot
    nchunk = nslot // 128
    assert nslot % 128 == 0

    cores = []
    for c in range(NCORES):
        es, dloc, en, cell = per_core[c]
        if negpad:
            idx = np.full(nslot, -1, dtype=np.int16)
        else:
            idx = np.zeros(nslot, dtype=np.int16)
        nrm = np.zeros(nslot, dtype=np.float32)
        dlo = np.zeros(nslot, dtype=np.float32)
        order = np.argsort(cell, kind="stable")
        cell_sorted = cell[order]
        # rank within cell
        cnt = counts[c]
        starts = np.zeros(ncell, dtype=np.int64)
        np.cumsum(cnt[:-1], out=starts[1:])
        rank = np.arange(len(order)) - starts[cell_sorted]
        pos = cell_slot0[cell_sorted] + rank
        idx[pos] = (es[order] - (cell_sorted % nb) * SUB).astype(np.int16)
        nrm[pos] = en[order]
        dlo[pos] = (dloc[order] & 127).astype(np.float32)
        # self-loop chunks: diag(dinv^2) over the tile's own rows
        base = c * npc
        dinv2 = (dinv[base:base + npc] ** 2).astype(np.float32)
        for (_, b, s0, ns) in batches:
            if b != -1:
                continue
            t = chunk_tile[s0 // 128]
            nr = min(128, npc - t * 128)
            dlo[s0:s0 + nr] = np.arange(nr, dtype=np.float32)
            nrm[s0:s0 + nr] = dinv2[t * 128:t * 128 + nr]
        # per-batch valid count for num_idxs_reg (ucode contract: reg ==
        # #non-negative idxs; all negatives trailing; at least one valid)
        bcnt = np.zeros(len(batches), dtype=np.int32)
        if negpad:
            for i, (_, b, s0, ns) in enumerate(batches):
                if b == -1:
                    bcnt[i] = 1  # unused (self chunks use plain DMA)
                    continue
                cl = chunk_tile[s0 // 128] * nb + b
                real = int(min(max(cnt[cl] - (s0 - cell_slot0[cl]), 0), ns))
                if real == 0:
                    idx[s0] = 0  # dummy valid gather; nrm stays 0
                    real = 1
                bcnt[i] = real
        else:
            for i, (_, _, s0, ns) in enumerate(batches):
                bcnt[i] = ns
        # wrap indices into 16 partitions, replicate to 128
        idx_t = np.ascontiguousarray(
            np.tile(idx.reshape(-1, 16).T, (8, 1)))  # [128, nslot//16]
        nrm_t = np.ascontiguousarray(nrm.reshape(-1, 128).T)  # [128, nchunk]
        dlo_t = np.ascontiguousarray(dlo.reshape(-1, 128).T)
        cores.append(dict(idx=idx_t, nrm=nrm_t, dlo=dlo_t, bcnt=bcnt))

    sched = dict(chunk_tile=chunk_tile, batches=batches, groups=groups,
                 nslot=nslot, nchunk=nchunk, ntile=ntile, nb=nb)
    return sched, cores


def _build_nc(cfg, sched, apply_bias, apply_g1b1, apply_g2b2, repeat=1,
              timing_mode=False):
    import concourse.bass as bass
    import concourse.bacc as bacc
    import concourse.mybir as mybir
    import concourse.tile as tile

    N, C, SUB, GRP = cfg["N"], cfg["C"], cfg["SUB"], cfg["GRP"]
    npc, ntile, npad, nb, ngrp = _derived(cfg)
    nslot, nchunk = sched["nslot"], sched["nchunk"]
    chunk_tile, batches, groups = (
        sched["chunk_tile"], sched["batches"], sched["groups"])
    f32, bf16, i16 = mybir.dt.float32, mybir.dt.bfloat16, mybir.dt.int16
    AF = mybir.ActivationFunctionType
    OP = mybir.AluOpType

    # first/last chunk index per psum bank (= up to 4 dst tiles of one
    # group); start=True zeroes a whole 2KB zero-region, so flags are
    # per bank
    tile_bank = {}
    for g, tiles in enumerate(groups):
        for t in tiles:
            tile_bank[t] = (g, (t - tiles[0]) // 4)
    first_ch, last_ch = {}, {}
    for q, t in enumerate(chunk_tile):
        bank = tile_bank[t]
        if bank not in first_ch:
            first_ch[bank] = q
        last_ch[bank] = q

    maxch = max(-(-ns // 128) for (_, b, _, ns) in batches if b >= 0)
    # widest (group, bucket) S run in slots, incl. per-group self blocks
    runspan = {}
    for (g, b, s0, ns) in batches:
        lo, hi = runspan.get((g, b), (s0, 0))
        runspan[(g, b)] = (min(lo, s0), max(hi, s0 + -(-ns // 128) * 128))
    smax = max(hi - lo for lo, hi in runspan.values())

    only_gather = cfg.get("ONLY_GATHER", False)
    no_gather = cfg.get("NO_GATHER", False)
    f32tab = cfg.get("F32TAB", False)
    spkt = cfg.get("SINGLE_PACKET", True)
    nqueues = cfg.get("QUEUES", 1)
    nc = bacc.Bacc("TRN2", target_bir_lowering=False, debug=False,
                   dynamic_dma_scratch_size=cfg.get("SCRATCH", 16384),
                   num_swdge_queues=nqueues)
    # timing_mode: only idx16 (drives gather addresses) stays external;
    # value-only tensors become internal DRAM so per-call host transfers
    # shrink from ~260MB to ~30MB
    big = "Internal" if timing_mode else "ExternalInput"
    gdt = f32 if f32tab else bf16
    hosts = cfg.get("HOSTS", 1)
    # xown/xownh/out use partition-major layout [128, ntile*C]: column t*C+f
    # of partition p holds node (t*128+p) feature f — a whole 4-tile half
    # then moves as one contiguous descriptor per partition instead of 128
    # short ones per tile
    xtab_d = nc.dram_tensor("xtab", [N, C], gdt, kind=big)
    xown_d = nc.dram_tensor("xown", [128, ntile * C], f32, kind=big)
    xownh_d = nc.dram_tensor("xownh", [128, ntile * C], gdt, kind=big)
    wt_d = nc.dram_tensor("wt", [C, C], f32, kind=big)
    iota_d = (None if hosts
              else nc.dram_tensor("iota", [128, 128], gdt, kind=big))
    idx_d = nc.dram_tensor("idx16", [128, nslot // 16], i16,
                           kind="ExternalInput")
    negpad = cfg.get("NEGPAD", 1)
    nbatch = len(batches)
    bcnt_d = (nc.dram_tensor("bcnt", [128, nbatch], mybir.dt.int32,
                             kind="ExternalInput") if negpad else None)
    sdat_d = (nc.dram_tensor("sdat", [128, nslot], gdt, kind=big)
              if hosts else None)
    nrm_d = (None if hosts
             else nc.dram_tensor("normT", [128, nchunk], f32, kind=big))
    dlo_d = (None if hosts
             else nc.dram_tensor("dstlocT", [128, nchunk], f32, kind=big))
    cvec_d = nc.dram_tensor("cvec", [128, 3 * C], f32, kind=big)
    out_d = nc.dram_tensor(
        "out", [128, ntile * C], f32,
        kind="Internal" if timing_mode else "ExternalOutput")
    dummy_d = (nc.dram_tensor("tdummy", [128, 1], f32, kind="ExternalOutput")
               if timing_mode else None)

    with tile.TileContext(nc) as tc:
        with (
            tc.tile_pool(name="const", bufs=1) as cpool,
            tc.tile_pool(name="gt", bufs=cfg.get("GBUFS", 6)) as gpool,
            tc.tile_pool(name="sS", bufs=cfg.get("SBUFS", 3)) as spool,
            tc.tile_pool(name="xs", bufs=2) as xpool,
            tc.tile_pool(name="work", bufs=3) as wpool,
            tc.tile_pool(name="stat", bufs=3) as stpool,
            tc.tile_pool(name="acc", bufs=cfg.get("ABUFS", 4),
                         space=bass.MemorySpace.PSUM) as apool,
            tc.tile_pool(name="ps2", bufs=2,
                         space=bass.MemorySpace.PSUM) as p2pool,
        ):
            wt_s = cpool.tile([C, C], f32)
            idx_s = cpool.tile([128, nslot // 16], i16)
            cvec_s = cpool.tile([128, 3 * C], f32)
            eps_s = cpool.tile([128, 1], f32)
            nc.gpsimd.memset(eps_s[:], float(EPS))
            nc.sync.dma_start(out=wt_s[:], in_=wt_d[:])
            nc.sync.dma_start(out=idx_s[:], in_=idx_d[:])
            nc.sync.dma_start(out=cvec_s[:], in_=cvec_d[:])
            if not hosts:
                iota_s = cpool.tile([128, 128], gdt)
                nrm_s = cpool.tile([128, nchunk], f32)
                dlo_s = cpool.tile([128, nchunk], f32)
                nc.sync.dma_start(out=iota_s[:], in_=iota_d[:])
                nc.sync.dma_start(out=nrm_s[:], in_=nrm_d[:])
                nc.sync.dma_start(out=dlo_s[:], in_=dlo_d[:])
            if negpad:
                bcnt_s = cpool.tile([128, nbatch], mybir.dt.int32)
                nc.sync.dma_start(out=bcnt_s[:], in_=bcnt_d[:])
                cnt_reg = nc.gpsimd.alloc_register("gcnt")
                # one-time zero of the gather pool: NEGPAD leaves pad slots
                # unwritten; recycled buffers then only ever hold finite
                # gathered rows, so 0-weight scatter rows stay NaN-free
                for _ in range(cfg.get("GBUFS", 6)):
                    zt = gpool.tile([128, maxch, 128], gdt, tag="gt")
                    nc.vector.memset(zt[:], 0.0)

            import contextlib
            loop_cm = (tc.For_i(0, repeat, 1) if repeat > 1
                       else contextlib.nullcontext())
            with loop_cm:
                q = 0  # global chunk cursor
                gather_i = 0
                for g, tiles in enumerate(groups):
                    t0 = tiles[0]
                    ntg = len(tiles)
                    acc = [apool.tile([128, 512], f32, tag="acc", name=f"acc{g}_{i}")
                           for i in range((ntg + 3) // 4)]
                    # gather + accumulate for this group, one bucket run at
                    # a time (S data for a run is contiguous -> one DMA)
                    gbatches = [(i, bt) for i, bt in enumerate(batches)
                                if bt[0] == g]
                    for rb in ([-1] + list(range(nb))):
                        rbatches = [(i, bt) for i, bt in gbatches
                                    if bt[1] == rb]
                        if not rbatches:
                            continue
                        run_s0 = rbatches[0][1][2]
                        run_end = max(bt[2] + -(-bt[3] // 128) * 128
                                      for _, bt in rbatches)
                        rw = run_end - run_s0
                        if hosts and not only_gather:
                            st = spool.tile([128, smax], gdt, tag="sS")
                            nc.sync.dma_start(
                                out=st[:, :rw],
                                in_=sdat_d[:, run_s0:run_end])
                        if rb < 0:
                            # self-loop chunks: own rows, one DMA per group
                            xs = xpool.tile([128, GRP, 128], gdt, tag="xs")
                            nc.sync.dma_start(
                                out=xs[:, :ntg, :],
                                in_=xownh_d[:, t0 * C:(t0 + ntg) * C])
                        for (bi, (_, b, s0, ns)) in rbatches:
                            nch = -(-ns // 128)
                            if b < 0:
                                gt_b = xs
                                ci0 = (s0 - run_s0) // 128
                            else:
                                gt = gpool.tile([128, maxch, 128], gdt,
                                                tag="gt")
                                gt_b = gt
                                ci0 = 0
                                win = min(N - b * SUB, SUB)
                                if ((negpad and cfg.get("SIMSAFE", 0))
                                        or no_gather):
                                    # CoreSim poisons fresh pool tiles with
                                    # NaN: sim runs re-zero pads per batch
                                    nc.vector.memset(gt[:, :nch, :], 0.0)
                                if not no_gather:
                                    if negpad:
                                        nc.gpsimd.reg_load(
                                            cnt_reg, bcnt_s[0:1, bi:bi + 1])
                                        nreg = cnt_reg
                                    else:
                                        nreg = ns
                                    nc.gpsimd.dma_gather(
                                        gt[:, :nch, :],
                                        xtab_d[b * SUB:b * SUB + win, :],
                                        idx_s[:, s0 // 16:
                                              s0 // 16 + -(-ns // 16)],
                                        num_idxs=ns,
                                        num_idxs_reg=nreg,
                                        elem_size=C,
                                        queue_num=gather_i % nqueues,
                                        single_packet=spkt,
                                    )
                                gather_i += 1
                            if only_gather:
                                q += nch
                                continue
                            for ci in range(nch):
                                t = chunk_tile[q]
                                if hosts:
                                    sc = q * 128 - run_s0
                                    S_ap = st[:, sc:sc + 128]
                                else:
                                    S = spool.tile([128, 128], gdt, tag="sS")
                                    nc.vector.tensor_scalar(
                                        out=S[:], in0=iota_s[:],
                                        scalar1=dlo_s[:, q:q + 1],
                                        scalar2=nrm_s[:, q:q + 1],
                                        op0=OP.is_equal, op1=OP.mult)
                                    S_ap = S[:]
                                j = t - t0
                                nc.tensor.matmul(
                                    acc[j // 4][:,
                                                (j % 4) * 128:
                                                (j % 4) * 128 + 128],
                                    gt_b[:, ci0 + ci, :], S_ap,
                                    start=(first_ch[tile_bank[t]] == q),
                                    stop=(last_ch[tile_bank[t]] == q))
                                q += 1
                    # transform + LN chain per 4-tile half
                    for h in range(0 if only_gather else (ntg + 3) // 4):
                        hw = min(4, ntg - h * 4)  # tiles in this half
                        W_ = hw * 128
                        aggT = wpool.tile([128, 512], f32, tag="aggT")
                        nc.vector.tensor_copy(aggT[:, :W_], acc[h][:, :W_])
                        ps2 = p2pool.tile([128, 512], f32, tag="ps2")
                        for j in range(hw):
                            nc.tensor.matmul(
                                ps2[:, j * 128:(j + 1) * 128],
                                aggT[:, j * 128:(j + 1) * 128], wt_s[:],
                                start=(j == 0), stop=(j == hw - 1))
                        h1 = wpool.tile([128, 4, 128], f32, tag="h1")
                        if apply_bias:
                            for j in range(hw):
                                nc.vector.tensor_tensor(
                                    out=h1[:, j, :],
                                    in0=ps2[:, j * 128:(j + 1) * 128],
                                    in1=cvec_s[:, 0:C], op=OP.add)
                            nc.scalar.activation(
                                out=h1[:, :hw, :], in_=h1[:, :hw, :], func=AF.Relu)
                        else:
                            nc.scalar.activation(
                                out=h1[:, :hw, :],
                                in_=ps2[:, :W_], func=AF.Relu)
                        xo = wpool.tile([128, 4, 128], f32, tag="xo")
                        tb = t0 + h * 4
                        nc.sync.dma_start(
                            out=xo[:, :hw, :],
                            in_=xown_d[:, tb * C:(tb + hw) * C])

                        def layer_norm(dst_t, src_t, gb_off):
                            # per-tile LN over the feature (free) dim
                            s1 = stpool.tile([128, 4], f32, tag="s1")
                            nmu = stpool.tile([128, 4], f32, tag="nmu")
                            ss = stpool.tile([128, 4], f32, tag="ss")
                            sq = wpool.tile([128, 4, 128], f32, tag="sq")
                            std = stpool.tile([128, 4], f32, tag="std")
                            rstd = stpool.tile([128, 4], f32, tag="rstd")
                            nc.vector.tensor_reduce(
                                out=s1[:, :hw], in_=src_t[:, :hw, :],
                                axis=mybir.AxisListType.X, op=OP.add)
                            nc.vector.tensor_scalar_mul(
                                nmu[:, :hw], s1[:, :hw], -1.0 / C)
                            for j in range(hw):
                                nc.scalar.activation(
                                    out=sq[:, j, :], in_=src_t[:, j, :],
                                    func=AF.Square, bias=nmu[:, j:j + 1],
                                    accum_out=ss[:, j:j + 1])
                            nc.scalar.activation(
                                out=std[:, :hw], in_=ss[:, :hw],
                                func=AF.Sqrt, bias=eps_s[:, 0:1], scale=1.0 / C)
                            nc.vector.reciprocal(rstd[:, :hw], std[:, :hw])
                            for j in range(hw):
                                nc.vector.tensor_scalar(
                                    out=dst_t[:, j, :], in0=src_t[:, j, :],
                                    scalar1=nmu[:, j:j + 1],
                                    scalar2=rstd[:, j:j + 1],
                                    op0=OP.add, op1=OP.mult)
                            if gb_off is not None:
                                for j in range(hw):
                                    nc.vector.tensor_tensor(
                                        out=dst_t[:, j, :], in0=dst_t[:, j, :],
                                        in1=cvec_s[:, gb_off:gb_off + C],
                                        op=OP.mult)
                                    nc.vector.tensor_tensor(
                                        out=dst_t[:, j, :], in0=dst_t[:, j, :],
                                        in1=cvec_s[:, gb_off + C:gb_off + 2 * C],
                                        op=OP.add)

                        y1 = wpool.tile([128, 4, 128], f32, tag="y1")
                        layer_norm(y1, h1, C if apply_g1b1 else None)
                        h2 = wpool.tile([128, 4, 128], f32, tag="h2")
                        nc.vector.tensor_tensor(
                            out=h2[:, :hw, :], in0=y1[:, :hw, :],
                            in1=xo[:, :hw, :], op=OP.add)
                        ot = wpool.tile([128, 4, 128], f32, tag="ot")
                        layer_norm(ot, h2, None)
                        if apply_g2b2:
                            # gamma2/beta2 live at cvec offset C (g1b1 unused then)
                            pass
                        nc.sync.dma_start(
                            out=out_d[:, tb * C:(tb + hw) * C],
                            in_=ot[:, :hw, :])
                assert q == nchunk
            if dummy_d is not None:
                nc.sync.dma_start(out=dummy_d[:], in_=eps_s[:])
    nc.compile()
    return nc


def _prep(cfg, x, edge_index, W, b, gamma1, beta1, gamma2, beta2):
    import ml_dtypes

    N, C, NCORES = cfg["N"], cfg["C"], cfg["NCORES"]
    npc, ntile, npad, nb, ngrp = _derived(cfg)
    src = np.asarray(edge_index[0], dtype=np.int64)
    dst = np.asarray(edge_index[1], dtype=np.int64)
    x = np.asarray(x, dtype=np.float32)
    W = np.asarray(W, dtype=np.float32)

    deg = (np.bincount(dst, minlength=N) + 1).astype(np.float32)
    dinv = (1.0 / np.sqrt(deg)).astype(np.float32)
    norm = (dinv[src] * dinv[dst]).astype(np.float32)

    sched, cores = _plan(cfg, src, dst, norm, dinv)

    gdt_np = np.float32 if cfg.get("F32TAB") else ml_dtypes.bfloat16
    hosts = cfg.get("HOSTS", 1)
    xtab = np.ascontiguousarray(x.astype(gdt_np))
    wt = np.ascontiguousarray(W.T).astype(np.float32)
    iota = np.ascontiguousarray(np.broadcast_to(
        np.arange(128, dtype=np.float32), (128, 128)).astype(gdt_np))
    cvec = np.zeros((128, 3 * C), dtype=np.float32)
    cvec[:, 0:C] = b
    cvec[:, C:2 * C] = gamma1
    cvec[:, 2 * C:3 * C] = beta1
    # (gamma2/beta2 identity assumed; asserted by caller flags)

    in_maps = []
    for c in range(NCORES):
        xo = np.zeros((npad, C), dtype=np.float32)
        xo[:npc] = x[c * npc:(c + 1) * npc]
        # partition-major: [128, ntile*C], col t*C+f <- node t*128+p
        xo2 = np.ascontiguousarray(
            xo.reshape(ntile, 128, C).transpose(1, 0, 2)
            .reshape(128, ntile * C))
        m = dict(
            xtab=xtab, xown=xo2, xownh=np.ascontiguousarray(
                xo2.astype(gdt_np)),
            wt=wt, idx16=cores[c]["idx"], cvec=cvec)
        if hosts:
            # dense one-hot scatter matrices, chunk-major: partition e holds
            # S_q[e, :] at cols [q*128, (q+1)*128)
            nslot = sched["nslot"]
            nchunk = sched["nchunk"]
            nrm = cores[c]["nrm"].T.reshape(-1)  # slot-order norms
            dlo = cores[c]["dlo"].T.reshape(-1).astype(np.int64)
            sall = np.zeros((nslot, 128), dtype=gdt_np)
            sall[np.arange(nslot), dlo] = nrm.astype(gdt_np)
            m["sdat"] = np.ascontiguousarray(
                sall.reshape(nchunk, 128, 128).transpose(1, 0, 2)
                .reshape(128, nslot))
        else:
            m["iota"] = iota
            m["normT"] = cores[c]["nrm"]
            m["dstlocT"] = cores[c]["dlo"]
        if cfg.get("NEGPAD", 1):
            m["bcnt"] = np.ascontiguousarray(
                np.broadcast_to(cores[c]["bcnt"], (128, len(cores[c]["bcnt"]))))
        in_maps.append(m)
    return sched, in_maps


def _run(cfg, sched, in_maps, apply_bias, apply_g1b1, apply_g2b2, **kw):
    import time

    from concourse.bass_utils import run_bass_kernel_spmd

    t0 = time.time()
    nc = _build_nc(cfg, sched, apply_bias, apply_g1b1, apply_g2b2)
    print(f"[kernel] build+tile-schedule: {time.time() - t0:.1f}s",
          flush=True)
    t0 = time.time()
    res = run_bass_kernel_spmd(
        nc, in_maps, list(range(cfg["NCORES"])), **kw)
    print(f"[kernel] compile+run: {time.time() - t0:.1f}s", flush=True)
    return nc, res


def kernel(x, edge_index, W, b, gamma1, beta1, gamma2, beta2,
           _profile_out=None):
    cfg = _cfg_full()
    N, C = cfg["N"], cfg["C"]
    npc, ntile, npad, nb, ngrp = _derived(cfg)
    apply_bias = bool(np.any(np.asarray(b)))
    apply_g1b1 = not (np.all(np.asarray(gamma1) == 1)
                      and not np.any(np.asarray(beta1)))
    apply_g2b2 = not (np.all(np.asarray(gamma2) == 1)
                      and not np.any(np.asarray(beta2)))
    assert not apply_g2b2, "general gamma2/beta2 not wired"
    sched, in_maps = _prep(cfg, x, edge_index, W, b,
                           gamma1, beta1, gamma2, beta2)
    kw = {}
    if _profile_out is not None:
        kw = dict(trace=True, tmpdir=_profile_out)
    nc, res = _run(cfg, sched, in_maps, apply_bias, apply_g1b1, apply_g2b2,
                   **kw)
    outs = []
    for c in range(cfg["NCORES"]):
        o2 = res.results[c]["out"]  # [128, ntile*C] partition-major
        o = o2.reshape(128, ntile, C).transpose(1, 0, 2).reshape(npad, C)
        outs.append(o[:npc])
    full = np.concatenate(outs, axis=0).astype(np.float32)
    if _profile_out is not None:
        return full, res
    return full



# revision 49
# speedup vs baseline: 1.0131x; 1.0131x over previous
"""GCN layer (PyG GCNConv + ReLU + LN + residual + LN) on 8 Trainium2 cores.

Math: out = LN2(x + LN1(relu(A_hat @ x @ W.T + b)))  with
A_hat = D^-1/2 (A+I) D^-1/2.  The per-edge weight factorizes
(norm_e = dinv[src]*dinv[dst]) and aggregation commutes with the linear
layer, so each core (nodes are dst-partitioned, hint-style):
  - gathers raw x rows (bf16) for the edges whose dst it owns (dma_gather)
  - scatter-adds them into per-dst-tile PSUM accumulators via one-hot
    matmuls on the PE: psumT[feat, node] += g_chunk.T @ S_chunk
  - applies W via a second matmul (psum2[node, feat] = aggT.T @ W.T)
  - runs the bias/relu/LN1/residual/LN2 chain on 512-wide tiles.

The kernel is gather-bound, and the gather is bound by SWDGE descriptor
generation on the Pool engine's Q7 cpus (~10ns per index per queue pair,
dominated by a scalar idx-copy loop in the ucode), not by HBM bytes.
Hence the key optimizations:
  - 4 SWDGE queues (each queue's desc-gen runs on its own Q7 cpu pair)
    with ~24 gather tiles in flight to keep all pairs fed
  - one gather instruction per (dst-tile, src-bucket) cell, trimmed to
    the max-over-cores edge count, with per-core trailing -1 idxs popped
    by the ucode (num_idxs_reg carries the per-core valid count from a
    small SBUF table) — pad slots cost no desc-gen or transfer
  - self-loops bypass the gather: per-tile diag(dinv^2) chunks whose own
    rows arrive via one contiguous DMA per group
  - scatter one-hot matrices S (dense bf16, 256B/edge) are precomputed
    on the host and streamed by bulk sequential DMA, one load per
    (group, bucket) run — removes ~2200 per-chunk DVE builds whose
    per-instruction overhead dominated compute
  - xown/out use partition-major DRAM layouts so residual loads and
    output stores move as one long descriptor per partition

Host-side numpy does graph preprocessing only: degrees, edge partitioning
by dst, bucketing by src>>15 (int16 gather-index windows), slot layout,
the S tables, and a static chunk schedule shared by all 8 cores
(per-core data rides in idx16/bcnt/sdat/xown tensors).
"""

import sys

import numpy as np

sys.path.insert(0, "/opt/trn_rl_repo")

EPS = 1e-5


def _cfg_full():
    return dict(
        N=100000,  # nodes
        C=128,  # features
        NCORES=8,
        SUB=32768,  # int16 gather window (rows per sub-table)
        GRP=8,  # dst tiles per psum group
        QUEUES=4,  # SWDGE queues (desc-gen runs on a Q7 cpu pair per queue)
        GBUFS=32,  # in-flight gather tiles
        SBUFS=4,  # in-flight scatter-matrix tiles
        NEGPAD=1,  # per-cell gather instructions + trailing -1 padding
        HOSTS=1,  # host-precomputed scatter one-hot matrices (DMA'd in)
        SIMSAFE=0,  # memset gather tiles every batch (CoreSim NaN poisoning)
        ABUFS=6,  # PSUM accumulator tiles (3 groups in flight)
    )


def _derived(cfg):
    N, NCORES = cfg["N"], cfg["NCORES"]
    npc = N // NCORES  # nodes per core
    assert npc * NCORES == N
    ntile = -(-npc // 128)  # dst tiles per core
    npad = ntile * 128
    nb = -(-N // cfg["SUB"])  # src buckets
    ngrp = -(-ntile // cfg["GRP"])
    return npc, ntile, npad, nb, ngrp


def _plan(cfg, src, dst, norm, dinv):
    """Build the shared static schedule + per-core host arrays.

    Returns (sched, cores) where sched has the chunk->tile mapping shared
    by all cores and cores[c] has idx/norm/dstloc arrays for core c.
    """
    N, C, NCORES, SUB, GRP = (
        cfg["N"], cfg["C"], cfg["NCORES"], cfg["SUB"], cfg["GRP"])
    npc, ntile, npad, nb, ngrp = _derived(cfg)
    ncell = ntile * nb

    per_core = []
    counts = np.zeros((NCORES, ncell), dtype=np.int64)
    for c in range(NCORES):
        base = c * npc
        m = (dst >= base) & (dst < base + npc)
        es, ed, en = src[m], dst[m], norm[m]
        # self loops (weight dinv[v]^2) are NOT routed through the gather:
        # they get a dedicated per-tile chunk loaded by plain contiguous DMA
        t = (ed - base) >> 7
        bkt = es // SUB
        cell = t * nb + bkt
        counts[c] = np.bincount(cell, minlength=ncell)
        per_core.append((es, ed - base, en, cell))

    cap = counts.max(axis=0)  # per (tile,bucket) max edges over cores
    chunks_per_cell = -(-cap // 128)  # 0 if cell empty on all cores
    # chunk schedule: group -> bucket -> tile in group -> chunks
    chunk_tile = []  # global chunk -> tile id
    cell_slot0 = np.zeros(ncell, dtype=np.int64)  # cell -> first slot
    batches = []  # (bucket, slot0, nslots) per gather instruction
    groups = []  # list of lists of tile ids
    bmax = cfg.get("BMAX", 896)
    negpad = cfg.get("NEGPAD", 1)
    slot = 0
    for g in range(ngrp):
        tiles = list(range(g * GRP, min((g + 1) * GRP, ntile)))
        groups.append(tiles)
        # dedicated self-loop chunk per tile: own rows arrive by plain
        # contiguous DMA (marker b=-1), scatter matrix is diag(dinv^2)
        for t in tiles:
            chunk_tile.append(t)
            batches.append((g, -1, slot, 128))
            slot += 128
        for b in range(nb):
            s0 = slot
            for t in tiles:
                cell = t * nb + b
                nch = int(chunks_per_cell[cell])
                if nch == 0:
                    continue
                cell_slot0[cell] = slot
                chunk_tile.extend([t] * nch)
                slot += nch * 128
                if negpad:
                    # per-cell instructions trimmed to the max-core count:
                    # the Q7 idx-copy loop costs ~10ns/idx, so the tail of
                    # 128-padding is dropped statically and per-core padding
                    # is dropped at runtime via trailing -1 idxs
                    p = slot - nch * 128
                    rem = int(cap[cell])
                    while rem > 0:
                        ns = min(bmax, rem)
                        batches.append((g, b, p, ns))
                        p += ns
                        rem -= ns
            if not negpad:
                # split into gather instructions of <= bmax indices (the
                # SWDGE descriptor carveout rejects much larger ones)
                p = s0
                while p < slot:
                    ns = min(bmax, slot - p)
                    batches.append((g, b, p, ns))
                    p += ns
    nslot = slot
    nchunk = nslot // 128
    assert nslot % 128 == 0

    cores = []
    for c in range(NCORES):
        es, dloc, en, cell = per_core[c]
        if negpad:
            idx = np.full(nslot, -1, dtype=np.int16)
        else:
            idx = np.zeros(nslot, dtype=np.int16)
        nrm = np.zeros(nslot, dtype=np.float32)
        dlo = np.zeros(nslot, dtype=np.float32)
        order = np.argsort(cell, kind="stable")
        cell_sorted = cell[order]
        # rank within cell
        cnt = counts[c]
        starts = np.zeros(ncell, dtype=np.int64)
        np.cumsum(cnt[:-1], out=starts[1:])
        rank = np.arange(len(order)) - starts[cell_sorted]
        pos = cell_slot0[cell_sorted] + rank
        idx[pos] = (es[order] - (cell_sorted % nb) * SUB).astype(np.int16)
        nrm[pos] = en[order]
        dlo[pos] = (dloc[order] & 127).astype(np.float32)
        # self-loop chunks: diag(dinv^2) over the tile's own rows
        base = c * npc
        dinv2 = (dinv[base:base + npc] ** 2).astype(np.float32)
        for (_, b, s0, ns) in batches:
            if b != -1:
                continue
            t = chunk_tile[s0 // 128]
            nr = min(128, npc - t * 128)
            dlo[s0:s0 + nr] = np.arange(nr, dtype=np.float32)
            nrm[s0:s0 + nr] = dinv2[t * 128:t * 128 + nr]
        # per-batch valid count for num_idxs_reg (ucode contract: reg ==
        # #non-negative idxs; all negatives trailing; at least one valid)
        bcnt = np.zeros(len(batches), dtype=np.int32)
        if negpad:
            for i, (_, b, s0, ns) in enumerate(batches):
                if b == -1:
                    bcnt[i] = 1  # unused (self chunks use plain DMA)
                    continue
                cl = chunk_tile[s0 // 128] * nb + b
                real = int(min(max(cnt[cl] - (s0 - cell_slot0[cl]), 0), ns))
                if real == 0:
                    idx[s0] = 0  # dummy valid gather; nrm stays 0
                    real = 1
                bcnt[i] = real
        else:
            for i, (_, _, s0, ns) in enumerate(batches):
                bcnt[i] = ns
        # wrap indices into 16 partitions, replicate to 128
        idx_t = np.ascontiguousarray(
            np.tile(idx.reshape(-1, 16).T, (8, 1)))  # [128, nslot//16]
        nrm_t = np.ascontiguousarray(nrm.reshape(-1, 128).T)  # [128, nchunk]
        dlo_t = np.ascontiguousarray(dlo.reshape(-1, 128).T)
        cores.append(dict(idx=idx_t, nrm=nrm_t, dlo=dlo_t, bcnt=bcnt))

    sched = dict(chunk_tile=chunk_tile, batches=batches, groups=groups,
                 nslot=nslot, nchunk=nchunk, ntile=ntile, nb=nb)
    return sched, cores


def _build_nc(cfg, sched, apply_bias, apply_g1b1, apply_g2b2, repeat=1,
              timing_mode=False):
    import concourse.bass as bass
    import concourse.bacc as bacc
    import concourse.mybir as mybir
    import concourse.tile as tile

    N, C, SUB, GRP = cfg["N"], cfg["C"], cfg["SUB"], cfg["GRP"]
    npc, ntile, npad, nb, ngrp = _derived(cfg)
    nslot, nchunk = sched["nslot"], sched["nchunk"]
    chunk_tile, batches, groups = (
        sched["chunk_tile"], sched["batches"], sched["groups"])
    f32, bf16, i16 = mybir.dt.float32, mybir.dt.bfloat16, mybir.dt.int16
    AF = mybir.ActivationFunctionType
    OP = mybir.AluOpType

    # first/last chunk index per psum bank (= up to 4 dst tiles of one
    # group); start=True zeroes a whole 2KB zero-region, so flags are
    # per bank
    tile_bank = {}
    for g, tiles in enumerate(groups):
        for t in tiles:
            tile_bank[t] = (g, (t - tiles[0]) // 4)
    first_ch, last_ch = {}, {}
    for q, t in enumerate(chunk_tile):
        bank = tile_bank[t]
        if bank not in first_ch:
            first_ch[bank] = q
        last_ch[bank] = q

    maxch = max(-(-ns // 128) for (_, b, _, ns) in batches if b >= 0)
    # widest (group, bucket) S run in slots, incl. per-group self blocks
    runspan = {}
    for (g, b, s0, ns) in batches:
        lo, hi = runspan.get((g, b), (s0, 0))
        runspan[(g, b)] = (min(lo, s0), max(hi, s0 + -(-ns // 128) * 128))
    smax = max(hi - lo for lo, hi in runspan.values())

    only_gather = cfg.get("ONLY_GATHER", False)
    no_gather = cfg.get("NO_GATHER", False)
    f32tab = cfg.get("F32TAB", False)
    spkt = cfg.get("SINGLE_PACKET", True)
    nqueues = cfg.get("QUEUES", 1)
    nc = bacc.Bacc("TRN2", target_bir_lowering=False, debug=False,
                   dynamic_dma_scratch_size=cfg.get("SCRATCH", 16384),
                   num_swdge_queues=nqueues)
    # timing_mode: only idx16 (drives gather addresses) stays external;
    # value-only tensors become internal DRAM so per-call host transfers
    # shrink from ~260MB to ~30MB
    big = "Internal" if timing_mode else "ExternalInput"
    gdt = f32 if f32tab else bf16
    hosts = cfg.get("HOSTS", 1)
    # xown/xownh/out use partition-major layout [128, ntile*C]: column t*C+f
    # of partition p holds node (t*128+p) feature f — a whole 4-tile half
    # then moves as one contiguous descriptor per partition instead of 128
    # short ones per tile
    xtab_d = nc.dram_tensor("xtab", [N, C], gdt, kind=big)
    xown_d = nc.dram_tensor("xown", [128, ntile * C], f32, kind=big)
    xownh_d = nc.dram_tensor("xownh", [128, ntile * C], gdt, kind=big)
    wt_d = nc.dram_tensor("wt", [C, C], f32, kind=big)
    iota_d = (None if hosts
              else nc.dram_tensor("iota", [128, 128], gdt, kind=big))
    idx_d = nc.dram_tensor("idx16", [128, nslot // 16], i16,
                           kind="ExternalInput")
    negpad = cfg.get("NEGPAD", 1)
    nbatch = len(batches)
    bcnt_d = (nc.dram_tensor("bcnt", [128, nbatch], mybir.dt.int32,
                             kind="ExternalInput") if negpad else None)
    sdat_d = (nc.dram_tensor("sdat", [128, nslot], gdt, kind=big)
              if hosts else None)
    nrm_d = (None if hosts
             else nc.dram_tensor("normT", [128, nchunk], f32, kind=big))
    dlo_d = (None if hosts
             else nc.dram_tensor("dstlocT", [128, nchunk], f32, kind=big))
    cvec_d = nc.dram_tensor("cvec", [128, 3 * C], f32, kind=big)
    out_d = nc.dram_tensor(
        "out", [128, ntile * C], f32,
        kind="Internal" if timing_mode else "ExternalOutput")
    dummy_d = (nc.dram_tensor("tdummy", [128, 1], f32, kind="ExternalOutput")
               if timing_mode else None)

    with tile.TileContext(nc) as tc:
        with (
            tc.tile_pool(name="const", bufs=1) as cpool,
            tc.tile_pool(name="gt", bufs=cfg.get("GBUFS", 6)) as gpool,
            tc.tile_pool(name="sS", bufs=cfg.get("SBUFS", 3)) as spool,
            tc.tile_pool(name="xs", bufs=2) as xpool,
            tc.tile_pool(name="work", bufs=3) as wpool,
            tc.tile_pool(name="stat", bufs=3) as stpool,
            tc.tile_pool(name="acc", bufs=cfg.get("ABUFS", 4),
                         space=bass.MemorySpace.PSUM) as apool,
            tc.tile_pool(name="ps2", bufs=2,
                         space=bass.MemorySpace.PSUM) as p2pool,
        ):
            wt_s = cpool.tile([C, C], f32)
            idx_s = cpool.tile([128, nslot // 16], i16)
            cvec_s = cpool.tile([128, 3 * C], f32)
            eps_s = cpool.tile([128, 1], f32)
            nc.gpsimd.memset(eps_s[:], float(EPS))
            nc.sync.dma_start(out=wt_s[:], in_=wt_d[:])
            nc.sync.dma_start(out=idx_s[:], in_=idx_d[:])
            nc.sync.dma_start(out=cvec_s[:], in_=cvec_d[:])
            if not hosts:
                iota_s = cpool.tile([128, 128], gdt)
                nrm_s = cpool.tile([128, nchunk], f32)
                dlo_s = cpool.tile([128, nchunk], f32)
                nc.sync.dma_start(out=iota_s[:], in_=iota_d[:])
                nc.sync.dma_start(out=nrm_s[:], in_=nrm_d[:])
                nc.sync.dma_start(out=dlo_s[:], in_=dlo_d[:])
            if negpad:
                bcnt_s = cpool.tile([128, nbatch], mybir.dt.int32)
                nc.sync.dma_start(out=bcnt_s[:], in_=bcnt_d[:])
                cnt_reg = nc.gpsimd.alloc_register("gcnt")
                # one-time zero of the gather pool: NEGPAD leaves pad slots
                # unwritten; recycled buffers then only ever hold finite
                # gathered rows, so 0-weight scatter rows stay NaN-free
                for _ in range(cfg.get("GBUFS", 6)):
                    zt = gpool.tile([128, maxch, 128], gdt, tag="gt")
                    nc.vector.memset(zt[:], 0.0)

            import contextlib
            loop_cm = (tc.For_i(0, repeat, 1) if repeat > 1
                       else contextlib.nullcontext())
            with loop_cm:
                q = 0  # global chunk cursor
                gather_i = 0
                for g, tiles in enumerate(groups):
                    t0 = tiles[0]
                    ntg = len(tiles)
                    acc = [apool.tile([128, 512], f32, tag="acc", name=f"acc{g}_{i}")
                           for i in range((ntg + 3) // 4)]
                    # gather + accumulate for this group, one bucket run at
                    # a time (S data for a run is contiguous -> one DMA)
                    gbatches = [(i, bt) for i, bt in enumerate(batches)
                                if bt[0] == g]
                    for rb in ([-1] + list(range(nb))):
                        rbatches = [(i, bt) for i, bt in gbatches
                                    if bt[1] == rb]
                        if not rbatches:
                            continue
                        run_s0 = rbatches[0][1][2]
                        run_end = max(bt[2] + -(-bt[3] // 128) * 128
                                      for _, bt in rbatches)
                        rw = run_end - run_s0
                        if hosts and not only_gather:
                            st = spool.tile([128, smax], gdt, tag="sS")
                            nc.sync.dma_start(
                                out=st[:, :rw],
                                in_=sdat_d[:, run_s0:run_end])
                        if rb < 0:
                            # self-loop chunks: own rows, one DMA per group
                            xs = xpool.tile([128, GRP, 128], gdt, tag="xs")
                            nc.sync.dma_start(
                                out=xs[:, :ntg, :],
                                in_=xownh_d[:, t0 * C:(t0 + ntg) * C])
                        for (bi, (_, b, s0, ns)) in rbatches:
                            nch = -(-ns // 128)
                            if b < 0:
                                gt_b = xs
                                ci0 = (s0 - run_s0) // 128
                            else:
                                gt = gpool.tile([128, maxch, 128], gdt,
                                                tag="gt")
                                gt_b = gt
                                ci0 = 0
                                win = min(N - b * SUB, SUB)
                                if ((negpad and cfg.get("SIMSAFE", 0))
                                        or no_gather):
                                    # CoreSim poisons fresh pool tiles with
                                    # NaN: sim runs re-zero pads per batch
                                    nc.vector.memset(gt[:, :nch, :], 0.0)
                                if not no_gather:
                                    if negpad:
                                        nc.gpsimd.reg_load(
                                            cnt_reg, bcnt_s[0:1, bi:bi + 1])
                                        nreg = cnt_reg
                                    else:
                                        nreg = ns
                                    nc.gpsimd.dma_gather(
                                        gt[:, :nch, :],
                                        xtab_d[b * SUB:b * SUB + win, :],
                                        idx_s[:, s0 // 16:
                                              s0 // 16 + -(-ns // 16)],
                                        num_idxs=ns,
                                        num_idxs_reg=nreg,
                                        elem_size=C,
                                        queue_num=gather_i % nqueues,
                                        single_packet=spkt,
                                    )
                                gather_i += 1
                            if only_gather:
                                q += nch
                                continue
                            for ci in range(nch):
                                t = chunk_tile[q]
                                if hosts:
                                    sc = q * 128 - run_s0
                                    S_ap = st[:, sc:sc + 128]
                                else:
                                    S = spool.tile([128, 128], gdt, tag="sS")
                                    nc.vector.tensor_scalar(
                                        out=S[:], in0=iota_s[:],
                                        scalar1=dlo_s[:, q:q + 1],
                                        scalar2=nrm_s[:, q:q + 1],
                                        op0=OP.is_equal, op1=OP.mult)
                                    S_ap = S[:]
                                j = t - t0
                                nc.tensor.matmul(
                                    acc[j // 4][:,
                                                (j % 4) * 128:
                                                (j % 4) * 128 + 128],
                                    gt_b[:, ci0 + ci, :], S_ap,
                                    start=(first_ch[tile_bank[t]] == q),
                                    stop=(last_ch[tile_bank[t]] == q))
                                q += 1
                    # transform + LN chain per 4-tile half
                    for h in range(0 if only_gather else (ntg + 3) // 4):
                        hw = min(4, ntg - h * 4)  # tiles in this half
                        W_ = hw * 128
                        aggT = wpool.tile([128, 512], f32, tag="aggT")
                        nc.vector.tensor_copy(aggT[:, :W_], acc[h][:, :W_])
                        ps2 = p2pool.tile([128, 512], f32, tag="ps2")
                        for j in range(hw):
                            nc.tensor.matmul(
                                ps2[:, j * 128:(j + 1) * 128],
                                aggT[:, j * 128:(j + 1) * 128], wt_s[:],
                                start=(j == 0), stop=(j == hw - 1))
                        h1 = wpool.tile([128, 4, 128], f32, tag="h1")
                        if apply_bias:
                            for j in range(hw):
                                nc.vector.tensor_tensor(
                                    out=h1[:, j, :],
                                    in0=ps2[:, j * 128:(j + 1) * 128],
                                    in1=cvec_s[:, 0:C], op=OP.add)
                            nc.scalar.activation(
                                out=h1[:, :hw, :], in_=h1[:, :hw, :], func=AF.Relu)
                        else:
                            nc.scalar.activation(
                                out=h1[:, :hw, :],
                                in_=ps2[:, :W_], func=AF.Relu)
                        xo = wpool.tile([128, 4, 128], f32, tag="xo")
                        tb = t0 + h * 4
                        nc.sync.dma_start(
                            out=xo[:, :hw, :],
                            in_=xown_d[:, tb * C:(tb + hw) * C])

                        def layer_norm(dst_t, src_t, gb_off):
                            # per-tile LN over the feature (free) dim
                            s1 = stpool.tile([128, 4], f32, tag="s1")
                            nmu = stpool.tile([128, 4], f32, tag="nmu")
                            ss = stpool.tile([128, 4], f32, tag="ss")
                            sq = wpool.tile([128, 4, 128], f32, tag="sq")
                            std = stpool.tile([128, 4], f32, tag="std")
                            rstd = stpool.tile([128, 4], f32, tag="rstd")
                            nc.vector.tensor_reduce(
                                out=s1[:, :hw], in_=src_t[:, :hw, :],
                                axis=mybir.AxisListType.X, op=OP.add)
                            nc.vector.tensor_scalar_mul(
                                nmu[:, :hw], s1[:, :hw], -1.0 / C)
                            for j in range(hw):
                                nc.scalar.activation(
                                    out=sq[:, j, :], in_=src_t[:, j, :],
                                    func=AF.Square, bias=nmu[:, j:j + 1],
                                    accum_out=ss[:, j:j + 1])
                            nc.scalar.activation(
                                out=std[:, :hw], in_=ss[:, :hw],
                                func=AF.Sqrt, bias=eps_s[:, 0:1], scale=1.0 / C)
                            nc.vector.reciprocal(rstd[:, :hw], std[:, :hw])
                            for j in range(hw):
                                nc.vector.tensor_scalar(
                                    out=dst_t[:, j, :], in0=src_t[:, j, :],
                                    scalar1=nmu[:, j:j + 1],
                                    scalar2=rstd[:, j:j + 1],
                                    op0=OP.add, op1=OP.mult)
                            if gb_off is not None:
                                for j in range(hw):
                                    nc.vector.tensor_tensor(
                                        out=dst_t[:, j, :], in0=dst_t[:, j, :],
                                        in1=cvec_s[:, gb_off:gb_off + C],
                                        op=OP.mult)
                                    nc.vector.tensor_tensor(
                                        out=dst_t[:, j, :], in0=dst_t[:, j, :],
                                        in1=cvec_s[:, gb_off + C:gb_off + 2 * C],
                                        op=OP.add)

                        y1 = wpool.tile([128, 4, 128], f32, tag="y1")
                        layer_norm(y1, h1, C if apply_g1b1 else None)
                        h2 = wpool.tile([128, 4, 128], f32, tag="h2")
                        nc.vector.tensor_tensor(
                            out=h2[:, :hw, :], in0=y1[:, :hw, :],
                            in1=xo[:, :hw, :], op=OP.add)
                        ot = wpool.tile([128, 4, 128], f32, tag="ot")
                        layer_norm(ot, h2, None)
                        if apply_g2b2:
                            # gamma2/beta2 live at cvec offset C (g1b1 unused then)
                            pass
                        nc.sync.dma_start(
                            out=out_d[:, tb * C:(tb + hw) * C],
                            in_=ot[:, :hw, :])
                assert q == nchunk
            if dummy_d is not None:
                nc.sync.dma_start(out=dummy_d[:], in_=eps_s[:])
    nc.compile()
    return nc


def _prep(cfg, x, edge_index, W, b, gamma1, beta1, gamma2, beta2):
    import ml_dtypes

    N, C, NCORES = cfg["N"], cfg["C"], cfg["NCORES"]
    npc, ntile, npad, nb, ngrp = _derived(cfg)
    src = np.asarray(edge_index[0], dtype=np.int64)
    dst = np.asarray(edge_index[1], dtype=np.int64)
    x = np.asarray(x, dtype=np.float32)
    W = np.asarray(W, dtype=np.float32)

    deg = (np.bincount(dst, minlength=N) + 1).astype(np.float32)
    dinv = (1.0 / np.sqrt(deg)).astype(np.float32)
    norm = (dinv[src] * dinv[dst]).astype(np.float32)

    sched, cores = _plan(cfg, src, dst, norm, dinv)

    gdt_np = np.float32 if cfg.get("F32TAB") else ml_dtypes.bfloat16
    hosts = cfg.get("HOSTS", 1)
    xtab = np.ascontiguousarray(x.astype(gdt_np))
    wt = np.ascontiguousarray(W.T).astype(np.float32)
    iota = np.ascontiguousarray(np.broadcast_to(
        np.arange(128, dtype=np.float32), (128, 128)).astype(gdt_np))
    cvec = np.zeros((128, 3 * C), dtype=np.float32)
    cvec[:, 0:C] = b
    cvec[:, C:2 * C] = gamma1
    cvec[:, 2 * C:3 * C] = beta1
    # (gamma2/beta2 identity assumed; asserted by caller flags)

    in_maps = []
    for c in range(NCORES):
        xo = np.zeros((npad, C), dtype=np.float32)
        xo[:npc] = x[c * npc:(c + 1) * npc]
        # partition-major: [128, ntile*C], col t*C+f <- node t*128+p
        xo2 = np.ascontiguousarray(
            xo.reshape(ntile, 128, C).transpose(1, 0, 2)
            .reshape(128, ntile * C))
        m = dict(
            xtab=xtab, xown=xo2, xownh=np.ascontiguousarray(
                xo2.astype(gdt_np)),
            wt=wt, idx16=cores[c]["idx"], cvec=cvec)
        if hosts:
            # dense one-hot scatter matrices, chunk-major: partition e holds
            # S_q[e, :] at cols [q*128, (q+1)*128)
            nslot = sched["nslot"]
            nchunk = sched["nchunk"]
            nrm = cores[c]["nrm"].T.reshape(-1)  # slot-order norms
            dlo = cores[c]["dlo"].T.reshape(-1).astype(np.int64)
            sall = np.zeros((nslot, 128), dtype=gdt_np)
            sall[np.arange(nslot), dlo] = nrm.astype(gdt_np)
            m["sdat"] = np.ascontiguousarray(
                sall.reshape(nchunk, 128, 128).transpose(1, 0, 2)
                .reshape(128, nslot))
        else:
            m["iota"] = iota
            m["normT"] = cores[c]["nrm"]
            m["dstlocT"] = cores[c]["dlo"]
        if cfg.get("NEGPAD", 1):
            m["bcnt"] = np.ascontiguousarray(
                np.broadcast_to(cores[c]["bcnt"], (128, len(cores[c]["bcnt"]))))
        in_maps.append(m)
    return sched, in_maps


def _run(cfg, sched, in_maps, apply_bias, apply_g1b1, apply_g2b2, **kw):
    import time

    from concourse.bass_utils import run_bass_kernel_spmd

    t0 = time.time()
    nc = _build_nc(cfg, sched, apply_bias, apply_g1b1, apply_g2b2)
    print(f"[kernel] build+tile-schedule: {time.time() - t0:.1f}s",
          flush=True)
    t0 = time.time()
    res = run_bass_kernel_spmd(
        nc, in_maps, list(range(cfg["NCORES"])), **kw)
    print(f"[kernel] compile+run: {time.time() - t0:.1f}s", flush=True)
    return nc, res


def kernel(x, edge_index, W, b, gamma1, beta1, gamma2, beta2,
           _profile_out=None):
    cfg = _cfg_full()
    N, C = cfg["N"], cfg["C"]
    npc, ntile, npad, nb, ngrp = _derived(cfg)
    apply_bias = bool(np.any(np.asarray(b)))
    apply_g1b1 = not (np.all(np.asarray(gamma1) == 1)
                      and not np.any(np.asarray(beta1)))
    apply_g2b2 = not (np.all(np.asarray(gamma2) == 1)
                      and not np.any(np.asarray(beta2)))
    assert not apply_g2b2, "general gamma2/beta2 not wired"
    sched, in_maps = _prep(cfg, x, edge_index, W, b,
                           gamma1, beta1, gamma2, beta2)
    kw = {}
    if _profile_out is not None:
        kw = dict(trace=True, tmpdir=_profile_out)
    nc, res = _run(cfg, sched, in_maps, apply_bias, apply_g1b1, apply_g2b2,
                   **kw)
    outs = []
    for c in range(cfg["NCORES"]):
        o2 = res.results[c]["out"]  # [128, ntile*C] partition-major
        o = o2.reshape(128, ntile, C).transpose(1, 0, 2).reshape(npad, C)
        outs.append(o[:npc])
    full = np.concatenate(outs, axis=0).astype(np.float32)
    if _profile_out is not None:
        return full, res
    return full



# revision 54
# speedup vs baseline: 1.0257x; 1.0124x over previous
"""GCN layer (PyG GCNConv + ReLU + LN + residual + LN) on 8 Trainium2 cores.

Math: out = LN2(x + LN1(relu(A_hat @ x @ W.T + b)))  with
A_hat = D^-1/2 (A+I) D^-1/2.  The per-edge weight factorizes
(norm_e = dinv[src]*dinv[dst]) and aggregation commutes with the linear
layer, so each core (nodes are dst-partitioned, hint-style):
  - gathers raw x rows (bf16) for the edges whose dst it owns (dma_gather)
  - scatter-adds them into per-dst-tile PSUM accumulators via one-hot
    matmuls on the PE: psumT[feat, node] += g_chunk.T @ S_chunk
  - applies W via a second matmul (psum2[node, feat] = aggT.T @ W.T)
  - runs the bias/relu/LN1/residual/LN2 chain on 512-wide tiles.

The kernel is gather-bound, and the gather is bound by SWDGE descriptor
generation on the Pool engine's Q7 cpus (~10ns per index per queue pair,
dominated by a scalar idx-copy loop in the ucode), not by HBM bytes.
Hence the key optimizations:
  - 4 SWDGE queues (each queue's desc-gen runs on its own Q7 cpu pair)
    with ~24 gather tiles in flight to keep all pairs fed
  - one gather instruction per (dst-tile, src-bucket) cell, trimmed to
    the max-over-cores edge count, with per-core trailing -1 idxs popped
    by the ucode (num_idxs_reg carries the per-core valid count from a
    small SBUF table) — pad slots cost no desc-gen or transfer
  - self-loops bypass the gather: per-tile diag(dinv^2) chunks whose own
    rows arrive via one contiguous DMA per group
  - scatter one-hot matrices S (dense bf16, 256B/edge) are precomputed
    on the host and streamed by bulk sequential DMA, one load per
    (group, bucket) run — removes ~2200 per-chunk DVE builds whose
    per-instruction overhead dominated compute
  - xown/out use partition-major DRAM layouts so residual loads and
    output stores move as one long descriptor per partition

Host-side numpy does graph preprocessing only: degrees, edge partitioning
by dst, bucketing by src>>15 (int16 gather-index windows), slot layout,
the S tables, and a static chunk schedule shared by all 8 cores
(per-core data rides in idx16/bcnt/sdat/xown tensors).
"""

import sys

import numpy as np

sys.path.insert(0, "/opt/trn_rl_repo")

EPS = 1e-5


def _cfg_full():
    return dict(
        N=100000,  # nodes
        C=128,  # features
        NCORES=8,
        SUB=32768,  # int16 gather window (rows per sub-table)
        GRP=8,  # dst tiles per psum group
        QUEUES=4,  # SWDGE queues (desc-gen runs on a Q7 cpu pair per queue)
        GBUFS=32,  # in-flight gather tiles
        SBUFS=4,  # in-flight scatter-matrix tiles
        NEGPAD=1,  # per-cell gather instructions + trailing -1 padding
        HOSTS=1,  # host-precomputed scatter one-hot matrices (DMA'd in)
        SIMSAFE=0,  # memset gather tiles every batch (CoreSim NaN poisoning)
        ABUFS=6,  # PSUM accumulator tiles (3 groups in flight)
    )


def _derived(cfg):
    N, NCORES = cfg["N"], cfg["NCORES"]
    npc = N // NCORES  # nodes per core
    assert npc * NCORES == N
    ntile = -(-npc // 128)  # dst tiles per core
    npad = ntile * 128
    nb = -(-N // cfg["SUB"])  # src buckets
    ngrp = -(-ntile // cfg["GRP"])
    return npc, ntile, npad, nb, ngrp


def _plan(cfg, src, dst, norm, dinv):
    """Build the shared static schedule + per-core host arrays.

    Returns (sched, cores) where sched has the chunk->tile mapping shared
    by all cores and cores[c] has idx/norm/dstloc arrays for core c.
    """
    N, C, NCORES, SUB, GRP = (
        cfg["N"], cfg["C"], cfg["NCORES"], cfg["SUB"], cfg["GRP"])
    npc, ntile, npad, nb, ngrp = _derived(cfg)
    ncell = ntile * nb

    per_core = []
    counts = np.zeros((NCORES, ncell), dtype=np.int64)
    for c in range(NCORES):
        base = c * npc
        m = (dst >= base) & (dst < base + npc)
        es, ed, en = src[m], dst[m], norm[m]
        # self loops (weight dinv[v]^2) are NOT routed through the gather:
        # they get a dedicated per-tile chunk loaded by plain contiguous DMA
        t = (ed - base) >> 7
        bkt = es // SUB
        cell = t * nb + bkt
        counts[c] = np.bincount(cell, minlength=ncell)
        per_core.append((es, ed - base, en, cell))

    cap = counts.max(axis=0)  # per (tile,bucket) max edges over cores
    chunks_per_cell = -(-cap // 128)  # 0 if cell empty on all cores
    # chunk schedule: group -> bucket -> tile in group -> chunks
    chunk_tile = []  # global chunk -> tile id
    cell_slot0 = np.zeros(ncell, dtype=np.int64)  # cell -> first slot
    batches = []  # (bucket, slot0, nslots) per gather instruction
    groups = []  # list of lists of tile ids
    bmax = cfg.get("BMAX", 896)
    negpad = cfg.get("NEGPAD", 1)
    slot = 0
    for g in range(ngrp):
        tiles = list(range(g * GRP, min((g + 1) * GRP, ntile)))
        groups.append(tiles)
        # dedicated self-loop chunk per tile: own rows arrive by plain
        # contiguous DMA (marker b=-1), scatter matrix is diag(dinv^2)
        for t in tiles:
            chunk_tile.append(t)
            batches.append((g, -1, slot, 128))
            slot += 128
        for b in range(nb):
            s0 = slot
            for t in tiles:
                cell = t * nb + b
                nch = int(chunks_per_cell[cell])
                if nch == 0:
                    continue
                cell_slot0[cell] = slot
                chunk_tile.extend([t] * nch)
                slot += nch * 128
                if negpad:
                    # per-cell instructions trimmed to the max-core count:
                    # the Q7 idx-copy loop costs ~10ns/idx, so the tail of
                    # 128-padding is dropped statically and per-core padding
                    # is dropped at runtime via trailing -1 idxs
                    p = slot - nch * 128
                    rem = int(cap[cell])
                    while rem > 0:
                        ns = min(bmax, rem)
                        batches.append((g, b, p, ns))
                        p += ns
                        rem -= ns
            if not negpad:
                # split into gather instructions of <= bmax indices (the
                # SWDGE descriptor carveout rejects much larger ones)
                p = s0
                while p < slot:
                    ns = min(bmax, slot - p)
                    batches.append((g, b, p, ns))
                    p += ns
    nslot = slot
    nchunk = nslot // 128
    assert nslot % 128 == 0

    cores = []
    for c in range(NCORES):
        es, dloc, en, cell = per_core[c]
        if negpad:
            idx = np.full(nslot, -1, dtype=np.int16)
        else:
            idx = np.zeros(nslot, dtype=np.int16)
        nrm = np.zeros(nslot, dtype=np.float32)
        dlo = np.zeros(nslot, dtype=np.float32)
        order = np.argsort(cell, kind="stable")
        cell_sorted = cell[order]
        # rank within cell
        cnt = counts[c]
        starts = np.zeros(ncell, dtype=np.int64)
        np.cumsum(cnt[:-1], out=starts[1:])
        rank = np.arange(len(order)) - starts[cell_sorted]
        pos = cell_slot0[cell_sorted] + rank
        idx[pos] = (es[order] - (cell_sorted % nb) * SUB).astype(np.int16)
        nrm[pos] = en[order]
        dlo[pos] = (dloc[order] & 127).astype(np.float32)
        # self-loop chunks: diag(dinv^2) over the tile's own rows
        base = c * npc
        dinv2 = (dinv[base:base + npc] ** 2).astype(np.float32)
        for (_, b, s0, ns) in batches:
            if b != -1:
                continue
            t = chunk_tile[s0 // 128]
            nr = min(128, npc - t * 128)
            dlo[s0:s0 + nr] = np.arange(nr, dtype=np.float32)
            nrm[s0:s0 + nr] = dinv2[t * 128:t * 128 + nr]
        # per-batch valid count for num_idxs_reg (ucode contract: reg ==
        # #non-negative idxs; all negatives trailing; at least one valid)
        bcnt = np.zeros(len(batches), dtype=np.int32)
        if negpad:
            for i, (_, b, s0, ns) in enumerate(batches):
                if b == -1:
                    bcnt[i] = 1  # unused (self chunks use plain DMA)
                    continue
                cl = chunk_tile[s0 // 128] * nb + b
                real = int(min(max(cnt[cl] - (s0 - cell_slot0[cl]), 0), ns))
                if real == 0:
                    idx[s0] = 0  # dummy valid gather; nrm stays 0
                    real = 1
                bcnt[i] = real
        else:
            for i, (_, _, s0, ns) in enumerate(batches):
                bcnt[i] = ns
        # wrap indices into 16 partitions, replicate to 128
        idx_t = np.ascontiguousarray(
            np.tile(idx.reshape(-1, 16).T, (8, 1)))  # [128, nslot//16]
        nrm_t = np.ascontiguousarray(nrm.reshape(-1, 128).T)  # [128, nchunk]
        dlo_t = np.ascontiguousarray(dlo.reshape(-1, 128).T)
        cores.append(dict(idx=idx_t, nrm=nrm_t, dlo=dlo_t, bcnt=bcnt))

    sched = dict(chunk_tile=chunk_tile, batches=batches, groups=groups,
                 nslot=nslot, nchunk=nchunk, ntile=ntile, nb=nb)
    return sched, cores


def _build_nc(cfg, sched, apply_bias, apply_g1b1, apply_g2b2, repeat=1,
              timing_mode=False):
    import concourse.bass as bass
    import concourse.bacc as bacc
    import concourse.mybir as mybir
    import concourse.tile as tile

    N, C, SUB, GRP = cfg["N"], cfg["C"], cfg["SUB"], cfg["GRP"]
    npc, ntile, npad, nb, ngrp = _derived(cfg)
    nslot, nchunk = sched["nslot"], sched["nchunk"]
    chunk_tile, batches, groups = (
        sched["chunk_tile"], sched["batches"], sched["groups"])
    f32, bf16, i16 = mybir.dt.float32, mybir.dt.bfloat16, mybir.dt.int16
    AF = mybir.ActivationFunctionType
    OP = mybir.AluOpType

    # first/last chunk index per psum bank (= up to 4 dst tiles of one
    # group); start=True zeroes a whole 2KB zero-region, so flags are
    # per bank
    tile_bank = {}
    for g, tiles in enumerate(groups):
        for t in tiles:
            tile_bank[t] = (g, (t - tiles[0]) // 4)
    first_ch, last_ch = {}, {}
    for q, t in enumerate(chunk_tile):
        bank = tile_bank[t]
        if bank not in first_ch:
            first_ch[bank] = q
        last_ch[bank] = q

    maxch = max(-(-ns // 128) for (_, b, _, ns) in batches if b >= 0)
    # widest (group, bucket) S run in slots, incl. per-group self blocks
    runspan = {}
    for (g, b, s0, ns) in batches:
        lo, hi = runspan.get((g, b), (s0, 0))
        runspan[(g, b)] = (min(lo, s0), max(hi, s0 + -(-ns // 128) * 128))
    smax = max(hi - lo for lo, hi in runspan.values())

    only_gather = cfg.get("ONLY_GATHER", False)
    no_gather = cfg.get("NO_GATHER", False)
    f32tab = cfg.get("F32TAB", False)
    spkt = cfg.get("SINGLE_PACKET", True)
    nqueues = cfg.get("QUEUES", 1)
    nc = bacc.Bacc("TRN2", target_bir_lowering=False, debug=False,
                   dynamic_dma_scratch_size=cfg.get("SCRATCH", 16384),
                   num_swdge_queues=nqueues)
    # timing_mode: only idx16 (drives gather addresses) stays external;
    # value-only tensors become internal DRAM so per-call host transfers
    # shrink from ~260MB to ~30MB
    big = "Internal" if timing_mode else "ExternalInput"
    gdt = f32 if f32tab else bf16
    hosts = cfg.get("HOSTS", 1)
    # xown/xownh/out use partition-major layout [128, ntile*C]: column t*C+f
    # of partition p holds node (t*128+p) feature f — a whole 4-tile half
    # then moves as one contiguous descriptor per partition instead of 128
    # short ones per tile
    xtab_d = nc.dram_tensor("xtab", [N, C], gdt, kind=big)
    xown_d = nc.dram_tensor("xown", [128, ntile * C], f32, kind=big)
    xownh_d = nc.dram_tensor("xownh", [128, ntile * C], gdt, kind=big)
    wt_d = nc.dram_tensor("wt", [C, C], f32, kind=big)
    iota_d = (None if hosts
              else nc.dram_tensor("iota", [128, 128], gdt, kind=big))
    idx_d = nc.dram_tensor("idx16", [128, nslot // 16], i16,
                           kind="ExternalInput")
    negpad = cfg.get("NEGPAD", 1)
    nbatch = len(batches)
    bcnt_d = (nc.dram_tensor("bcnt", [128, nbatch], mybir.dt.int32,
                             kind="ExternalInput") if negpad else None)
    bcs = cfg.get("BCS", 0)
    sdat_d = (nc.dram_tensor("sdat", [128, nslot], gdt, kind=big)
              if hosts and not bcs else None)
    dloh_d = (nc.dram_tensor("dloh", [128, nchunk], bf16, kind=big)
              if bcs else None)
    nrmh_d = (nc.dram_tensor("nrmh", [128, nchunk], bf16, kind=big)
              if bcs else None)
    iotb_d = (nc.dram_tensor("iotb", [128, 128], gdt, kind=big)
              if bcs else None)
    nrm_d = (None if hosts
             else nc.dram_tensor("normT", [128, nchunk], f32, kind=big))
    dlo_d = (None if hosts
             else nc.dram_tensor("dstlocT", [128, nchunk], f32, kind=big))
    cvec_d = nc.dram_tensor("cvec", [128, 3 * C], f32, kind=big)
    out_d = nc.dram_tensor(
        "out", [128, ntile * C], f32,
        kind="Internal" if timing_mode else "ExternalOutput")
    dummy_d = (nc.dram_tensor("tdummy", [128, 1], f32, kind="ExternalOutput")
               if timing_mode else None)

    with tile.TileContext(nc) as tc:
        with (
            tc.tile_pool(name="const", bufs=1) as cpool,
            tc.tile_pool(name="gt", bufs=cfg.get("GBUFS", 6)) as gpool,
            tc.tile_pool(name="sS", bufs=cfg.get("SBUFS", 3)) as spool,
            tc.tile_pool(name="xs", bufs=2) as xpool,
            tc.tile_pool(name="work", bufs=3) as wpool,
            tc.tile_pool(name="stat", bufs=3) as stpool,
            tc.tile_pool(name="acc", bufs=cfg.get("ABUFS", 4),
                         space=bass.MemorySpace.PSUM) as apool,
            tc.tile_pool(name="ps2", bufs=2,
                         space=bass.MemorySpace.PSUM) as p2pool,
        ):
            wt_s = cpool.tile([C, C], f32)
            idx_s = cpool.tile([128, nslot // 16], i16)
            cvec_s = cpool.tile([128, 3 * C], f32)
            eps_s = cpool.tile([128, 1], f32)
            nc.gpsimd.memset(eps_s[:], float(EPS))
            nc.sync.dma_start(out=wt_s[:], in_=wt_d[:])
            nc.sync.dma_start(out=idx_s[:], in_=idx_d[:])
            nc.sync.dma_start(out=cvec_s[:], in_=cvec_d[:])
            if not hosts:
                iota_s = cpool.tile([128, 128], gdt)
                nrm_s = cpool.tile([128, nchunk], f32)
                dlo_s = cpool.tile([128, nchunk], f32)
                nc.sync.dma_start(out=iota_s[:], in_=iota_d[:])
                nc.sync.dma_start(out=nrm_s[:], in_=nrm_d[:])
                nc.sync.dma_start(out=dlo_s[:], in_=dlo_d[:])
            if bcs:
                dloh_s = cpool.tile([128, nchunk], bf16)
                nrmh_s = cpool.tile([128, nchunk], bf16)
                iotb_s = cpool.tile([128, 128], gdt)
                nc.sync.dma_start(out=dloh_s[:], in_=dloh_d[:])
                nc.sync.dma_start(out=nrmh_s[:], in_=nrmh_d[:])
                nc.sync.dma_start(out=iotb_s[:], in_=iotb_d[:])
            if negpad:
                bcnt_s = cpool.tile([128, nbatch], mybir.dt.int32)
                nc.sync.dma_start(out=bcnt_s[:], in_=bcnt_d[:])
                cnt_reg = nc.gpsimd.alloc_register("gcnt")
                # one-time zero of the gather pool: NEGPAD leaves pad slots
                # unwritten; recycled buffers then only ever hold finite
                # gathered rows, so 0-weight scatter rows stay NaN-free
                for _ in range(cfg.get("GBUFS", 6)):
                    zt = gpool.tile([128, maxch, 128], gdt, tag="gt")
                    nc.vector.memset(zt[:], 0.0)

            import contextlib
            loop_cm = (tc.For_i(0, repeat, 1) if repeat > 1
                       else contextlib.nullcontext())
            with loop_cm:
                q = 0  # global chunk cursor
                gather_i = 0
                for g, tiles in enumerate(groups):
                    t0 = tiles[0]
                    ntg = len(tiles)
                    acc = [apool.tile([128, 512], f32, tag="acc", name=f"acc{g}_{i}")
                           for i in range((ntg + 3) // 4)]
                    # gather + accumulate for this group, one bucket run at
                    # a time (S data for a run is contiguous -> one DMA)
                    gbatches = [(i, bt) for i, bt in enumerate(batches)
                                if bt[0] == g]
                    for rb in ([-1] + list(range(nb))):
                        rbatches = [(i, bt) for i, bt in gbatches
                                    if bt[1] == rb]
                        if not rbatches:
                            continue
                        run_s0 = rbatches[0][1][2]
                        run_end = max(bt[2] + -(-bt[3] // 128) * 128
                                      for _, bt in rbatches)
                        rw = run_end - run_s0
                        if hosts and not only_gather:
                            st = spool.tile([128, smax], gdt, tag="sS")
                            if bcs:
                                # build S on-chip: 2 DVE ops per run using
                                # 0-stride broadcast APs over per-chunk
                                # dstloc/norm (d-iota repeated per chunk)
                                K_ = rw // 128
                                k0 = run_s0 // 128
                                o3 = st[:, :rw]
                                o3 = bass.AP(o3.tensor, o3.offset,
                                             [o3.ap[0], [128, K_], [1, 128]])
                                dv = dloh_s[:, k0:k0 + K_]
                                dv = bass.AP(dv.tensor, dv.offset,
                                             [dv.ap[0], [1, K_], [0, 128]])
                                nv = nrmh_s[:, k0:k0 + K_]
                                nv = bass.AP(nv.tensor, nv.offset,
                                             [nv.ap[0], [1, K_], [0, 128]])
                                it = iotb_s[:, :]
                                it = bass.AP(it.tensor, it.offset,
                                             [it.ap[0], [0, K_], [1, 128]])
                                nc.vector.tensor_tensor(
                                    out=o3, in0=dv, in1=it, op=OP.is_equal)
                                nc.vector.tensor_tensor(
                                    out=o3, in0=o3, in1=nv, op=OP.mult)
                            else:
                                nc.sync.dma_start(
                                    out=st[:, :rw],
                                    in_=sdat_d[:, run_s0:run_end])
                        if rb < 0:
                            # self-loop chunks: own rows, one DMA per group
                            xs = xpool.tile([128, GRP, 128], gdt, tag="xs")
                            nc.sync.dma_start(
                                out=xs[:, :ntg, :],
                                in_=xownh_d[:, t0 * C:(t0 + ntg) * C])
                        for (bi, (_, b, s0, ns)) in rbatches:
                            nch = -(-ns // 128)
                            if b < 0:
                                gt_b = xs
                                ci0 = (s0 - run_s0) // 128
                            else:
                                gt = gpool.tile([128, maxch, 128], gdt,
                                                tag="gt")
                                gt_b = gt
                                ci0 = 0
                                win = min(N - b * SUB, SUB)
                                if ((negpad and cfg.get("SIMSAFE", 0))
                                        or no_gather):
                                    # CoreSim poisons fresh pool tiles with
                                    # NaN: sim runs re-zero pads per batch
                                    nc.vector.memset(gt[:, :nch, :], 0.0)
                                if not no_gather:
                                    if negpad:
                                        nc.gpsimd.reg_load(
                                            cnt_reg, bcnt_s[0:1, bi:bi + 1])
                                        nreg = cnt_reg
                                    else:
                                        nreg = ns
                                    nc.gpsimd.dma_gather(
                                        gt[:, :nch, :],
                                        xtab_d[b * SUB:b * SUB + win, :],
                                        idx_s[:, s0 // 16:
                                              s0 // 16 + -(-ns // 16)],
                                        num_idxs=ns,
                                        num_idxs_reg=nreg,
                                        elem_size=C,
                                        queue_num=gather_i % nqueues,
                                        single_packet=spkt,
                                    )
                                gather_i += 1
                            if only_gather:
                                q += nch
                                continue
                            for ci in range(nch):
                                t = chunk_tile[q]
                                if hosts:
                                    sc = q * 128 - run_s0
                                    S_ap = st[:, sc:sc + 128]
                                else:
                                    S = spool.tile([128, 128], gdt, tag="sS")
                                    nc.vector.tensor_scalar(
                                        out=S[:], in0=iota_s[:],
                                        scalar1=dlo_s[:, q:q + 1],
                                        scalar2=nrm_s[:, q:q + 1],
                                        op0=OP.is_equal, op1=OP.mult)
                                    S_ap = S[:]
                                j = t - t0
                                nc.tensor.matmul(
                                    acc[j // 4][:,
                                                (j % 4) * 128:
                                                (j % 4) * 128 + 128],
                                    gt_b[:, ci0 + ci, :], S_ap,
                                    start=(first_ch[tile_bank[t]] == q),
                                    stop=(last_ch[tile_bank[t]] == q))
                                q += 1
                    # transform + LN chain per 4-tile half
                    for h in range(0 if only_gather else (ntg + 3) // 4):
                        hw = min(4, ntg - h * 4)  # tiles in this half
                        W_ = hw * 128
                        aggT = wpool.tile([128, 512], f32, tag="aggT")
                        nc.vector.tensor_copy(aggT[:, :W_], acc[h][:, :W_])
                        ps2 = p2pool.tile([128, 512], f32, tag="ps2")
                        for j in range(hw):
                            nc.tensor.matmul(
                                ps2[:, j * 128:(j + 1) * 128],
                                aggT[:, j * 128:(j + 1) * 128], wt_s[:],
                                start=(j == 0), stop=(j == hw - 1))
                        h1 = wpool.tile([128, 4, 128], f32, tag="h1")
                        if apply_bias:
                            for j in range(hw):
                                nc.vector.tensor_tensor(
                                    out=h1[:, j, :],
                                    in0=ps2[:, j * 128:(j + 1) * 128],
                                    in1=cvec_s[:, 0:C], op=OP.add)
                            nc.scalar.activation(
                                out=h1[:, :hw, :], in_=h1[:, :hw, :], func=AF.Relu)
                        else:
                            nc.scalar.activation(
                                out=h1[:, :hw, :],
                                in_=ps2[:, :W_], func=AF.Relu)
                        xo = wpool.tile([128, 4, 128], f32, tag="xo")
                        tb = t0 + h * 4
                        nc.sync.dma_start(
                            out=xo[:, :hw, :],
                            in_=xown_d[:, tb * C:(tb + hw) * C])

                        def layer_norm(dst_t, src_t, gb_off):
                            # per-tile LN over the feature (free) dim
                            s1 = stpool.tile([128, 4], f32, tag="s1")
                            nmu = stpool.tile([128, 4], f32, tag="nmu")
                            ss = stpool.tile([128, 4], f32, tag="ss")
                            sq = wpool.tile([128, 4, 128], f32, tag="sq")
                            std = stpool.tile([128, 4], f32, tag="std")
                            rstd = stpool.tile([128, 4], f32, tag="rstd")
                            nc.vector.tensor_reduce(
                                out=s1[:, :hw], in_=src_t[:, :hw, :],
                                axis=mybir.AxisListType.X, op=OP.add)
                            nc.vector.tensor_scalar_mul(
                                nmu[:, :hw], s1[:, :hw], -1.0 / C)
                            for j in range(hw):
                                nc.scalar.activation(
                                    out=sq[:, j, :], in_=src_t[:, j, :],
                                    func=AF.Square, bias=nmu[:, j:j + 1],
                                    accum_out=ss[:, j:j + 1])
                            nc.scalar.activation(
                                out=std[:, :hw], in_=ss[:, :hw],
                                func=AF.Sqrt, bias=eps_s[:, 0:1], scale=1.0 / C)
                            nc.vector.reciprocal(rstd[:, :hw], std[:, :hw])
                            for j in range(hw):
                                nc.vector.tensor_scalar(
                                    out=dst_t[:, j, :], in0=src_t[:, j, :],
                                    scalar1=nmu[:, j:j + 1],
                                    scalar2=rstd[:, j:j + 1],
                                    op0=OP.add, op1=OP.mult)
                            if gb_off is not None:
                                for j in range(hw):
                                    nc.vector.tensor_tensor(
                                        out=dst_t[:, j, :], in0=dst_t[:, j, :],
                                        in1=cvec_s[:, gb_off:gb_off + C],
                                        op=OP.mult)
                                    nc.vector.tensor_tensor(
                                        out=dst_t[:, j, :], in0=dst_t[:, j, :],
                                        in1=cvec_s[:, gb_off + C:gb_off + 2 * C],
                                        op=OP.add)

                        y1 = wpool.tile([128, 4, 128], f32, tag="y1")
                        layer_norm(y1, h1, C if apply_g1b1 else None)
                        h2 = wpool.tile([128, 4, 128], f32, tag="h2")
                        nc.vector.tensor_tensor(
                            out=h2[:, :hw, :], in0=y1[:, :hw, :],
                            in1=xo[:, :hw, :], op=OP.add)
                        ot = wpool.tile([128, 4, 128], f32, tag="ot")
                        layer_norm(ot, h2, None)
                        if apply_g2b2:
                            # gamma2/beta2 live at cvec offset C (g1b1 unused then)
                            pass
                        nc.sync.dma_start(
                            out=out_d[:, tb * C:(tb + hw) * C],
                            in_=ot[:, :hw, :])
                assert q == nchunk
            if dummy_d is not None:
                nc.sync.dma_start(out=dummy_d[:], in_=eps_s[:])
    nc.compile()
    return nc


def _prep(cfg, x, edge_index, W, b, gamma1, beta1, gamma2, beta2):
    import ml_dtypes

    N, C, NCORES = cfg["N"], cfg["C"], cfg["NCORES"]
    npc, ntile, npad, nb, ngrp = _derived(cfg)
    src = np.asarray(edge_index[0], dtype=np.int64)
    dst = np.asarray(edge_index[1], dtype=np.int64)
    x = np.asarray(x, dtype=np.float32)
    W = np.asarray(W, dtype=np.float32)

    deg = (np.bincount(dst, minlength=N) + 1).astype(np.float32)
    dinv = (1.0 / np.sqrt(deg)).astype(np.float32)
    norm = (dinv[src] * dinv[dst]).astype(np.float32)

    sched, cores = _plan(cfg, src, dst, norm, dinv)

    gdt_np = np.float32 if cfg.get("F32TAB") else ml_dtypes.bfloat16
    hosts = cfg.get("HOSTS", 1)
    xtab = np.ascontiguousarray(x.astype(gdt_np))
    wt = np.ascontiguousarray(W.T).astype(np.float32)
    iota = np.ascontiguousarray(np.broadcast_to(
        np.arange(128, dtype=np.float32), (128, 128)).astype(gdt_np))
    cvec = np.zeros((128, 3 * C), dtype=np.float32)
    cvec[:, 0:C] = b
    cvec[:, C:2 * C] = gamma1
    cvec[:, 2 * C:3 * C] = beta1
    # (gamma2/beta2 identity assumed; asserted by caller flags)

    in_maps = []
    for c in range(NCORES):
        xo = np.zeros((npad, C), dtype=np.float32)
        xo[:npc] = x[c * npc:(c + 1) * npc]
        # partition-major: [128, ntile*C], col t*C+f <- node t*128+p
        xo2 = np.ascontiguousarray(
            xo.reshape(ntile, 128, C).transpose(1, 0, 2)
            .reshape(128, ntile * C))
        m = dict(
            xtab=xtab, xown=xo2, xownh=np.ascontiguousarray(
                xo2.astype(gdt_np)),
            wt=wt, idx16=cores[c]["idx"], cvec=cvec)
        if hosts and cfg.get("BCS", 0):
            m["dloh"] = np.ascontiguousarray(
                cores[c]["dlo"].astype(ml_dtypes.bfloat16))
            m["nrmh"] = np.ascontiguousarray(
                cores[c]["nrm"].astype(ml_dtypes.bfloat16))
            m["iotb"] = iota
        elif hosts:
            # dense one-hot scatter matrices, chunk-major: partition e holds
            # S_q[e, :] at cols [q*128, (q+1)*128)
            nslot = sched["nslot"]
            nchunk = sched["nchunk"]
            nrm = cores[c]["nrm"].T.reshape(-1)  # slot-order norms
            dlo = cores[c]["dlo"].T.reshape(-1).astype(np.int64)
            sall = np.zeros((nslot, 128), dtype=gdt_np)
            sall[np.arange(nslot), dlo] = nrm.astype(gdt_np)
            m["sdat"] = np.ascontiguousarray(
                sall.reshape(nchunk, 128, 128).transpose(1, 0, 2)
                .reshape(128, nslot))
        else:
            m["iota"] = iota
            m["normT"] = cores[c]["nrm"]
            m["dstlocT"] = cores[c]["dlo"]
        if cfg.get("NEGPAD", 1):
            m["bcnt"] = np.ascontiguousarray(
                np.broadcast_to(cores[c]["bcnt"], (128, len(cores[c]["bcnt"]))))
        in_maps.append(m)
    return sched, in_maps


def _run(cfg, sched, in_maps, apply_bias, apply_g1b1, apply_g2b2, **kw):
    import time

    from concourse.bass_utils import run_bass_kernel_spmd

    t0 = time.time()
    nc = _build_nc(cfg, sched, apply_bias, apply_g1b1, apply_g2b2)
    print(f"[kernel] build+tile-schedule: {time.time() - t0:.1f}s",
          flush=True)
    t0 = time.time()
    res = run_bass_kernel_spmd(
        nc, in_maps, list(range(cfg["NCORES"])), **kw)
    print(f"[kernel] compile+run: {time.time() - t0:.1f}s", flush=True)
    return nc, res


def kernel(x, edge_index, W, b, gamma1, beta1, gamma2, beta2,
           _profile_out=None):
    cfg = _cfg_full()
    N, C = cfg["N"], cfg["C"]
    npc, ntile, npad, nb, ngrp = _derived(cfg)
    apply_bias = bool(np.any(np.asarray(b)))
    apply_g1b1 = not (np.all(np.asarray(gamma1) == 1)
                      and not np.any(np.asarray(beta1)))
    apply_g2b2 = not (np.all(np.asarray(gamma2) == 1)
                      and not np.any(np.asarray(beta2)))
    assert not apply_g2b2, "general gamma2/beta2 not wired"
    sched, in_maps = _prep(cfg, x, edge_index, W, b,
                           gamma1, beta1, gamma2, beta2)
    kw = {}
    if _profile_out is not None:
        kw = dict(trace=True, tmpdir=_profile_out)
    nc, res = _run(cfg, sched, in_maps, apply_bias, apply_g1b1, apply_g2b2,
                   **kw)
    outs = []
    for c in range(cfg["NCORES"]):
        o2 = res.results[c]["out"]  # [128, ntile*C] partition-major
        o = o2.reshape(128, ntile, C).transpose(1, 0, 2).reshape(npad, C)
        outs.append(o[:npc])
    full = np.concatenate(outs, axis=0).astype(np.float32)
    if _profile_out is not None:
        return full, res
    return full

